# revision 6
# baseline (speedup 1.0000x reference)
"""Trainium2 Bass kernel for nn_BilinearHead (RMSNorm -> two 1x1 convs ->
bilinear scores at fixed index pairs + promo bias).

Math (per batch b):
    rms2[b]    = mean(x[b]**2) + eps
    f[b]       = from_w @ (x[b] * norm_weight) ;  t[b] = to_w @ (...)
    score[b,v] = <f[b,:,from_idx[v]], t[b,:,to_idx[v]]> / rms2[b]
                 + promo_bias[promo_idx[v]]
(valid because norm_weight == 1 and the conv biases are 0 for this problem's
input distribution; kernel() verifies and falls back to a host reference
otherwise).

Device algorithm (pure data parallel over batch: 8 cores x 128 batches),
all-fp16 on device:

  1. Host pre-packs x as fp16 [cp=128, b=128, par=2, hw=64] (4KB contiguous
     per partition per group DMA) and wpack as [cp, 4, 2, 128] (contiguous
     2KB/partition DMA -- a scattered layout here cost ~5us and gated the
     first GEMM in earlier versions).
  2. x group loads are spread across the sync/scalar HWDGE queues and the
     gpsimd SWDGE queue, all issued at the head so DMA streams continuously.
  3. Per batch-group of 16: one DVE square over [128, 2048], then the
     per-(cp,b) reduction split between GPSIMD (otherwise idle) and DVE.
  4. PE GEMM (fp16, parity-packed stacked weights): psum rows 0-63 =
     even-batch d, 64-127 = odd-batch d -> f, t in adjacent psum banks;
     single full-lane ACT evict.
  5. PE pair-packed Gt matmuls (quadrants (0,0)/(64,0), separate psum
     banks) -> ACT-evict to gt[64 j, 128 b, 64 i].
  6. After the loop: PE transpose z -> DVE reduce/scale/recip -> inv[b],
     overlapping the score matmuls.
  7. PE one-hot score matmuls per distinct from_idx value (columns sorted
     by from_idx); fused finalize per psum chunk on DVE
     (scalar_tensor_tensor: score * inv[b] + promo) -> fp16 -> DMA out.
  8. Host un-sorts columns and casts fp32.
"""

import sys

sys.path.insert(0, "/opt/trn_rl_repo")

import numpy as np

import concourse.bass as bass
import concourse.tile as tile
from concourse import mybir
from concourse.bacc import Bacc
from concourse.bass_utils import run_bass_kernel_spmd

# Problem shape (hardcoded per contest contract)
B_TOT, C, HW, D, V = 1024, 256, 64, 64, 1968
N_CORES = 8
B = B_TOT // N_CORES  # 128 batches per core
CP = C // 2  # 128 channel pairs (partition dim for GEMM)
NGROUPS = 8
GB = B // NGROUPS  # 16 batches per group
PAIRS_PER_GROUP = GB // 2
EPS = 1e-6
# score psum chunk boundaries (<=512 per bank); the last chunk is tiny so
# the final finalize+store tail after the last score matmul is short
CHUNK_BOUNDS = [0, 512, 1024, 1536, 1904, V]
F32 = mybir.dt.float32
F16 = mybir.dt.float16

# ---- engine-assignment knobs ----
# x^2 path: DVE squares the whole group, GPSIMD (otherwise idle) does the
# first fold-add (GPSIMD has no free-axis reduce, but tensor_add works),
# DVE reduces the folded half. GP_F1_B = batches folded on GPSIMD.
GP_F1_B = 16
# split the last group's evictions across ACT and DVE to cut the
# end-of-loop latency chain
SPLIT_LAST = False
# warmup matmuls (64-col) to lift the HAM clock gate while x0 loads
WARMUP_MM = 44


def build_kernel(seg_plan):
    """seg_plan: list of (i, col0, ncols) score-matmul segments, where i is
    the from_idx value, col0 the starting column in from_idx-sorted order,
    and the segment does not cross a 512 psum-bank boundary."""
    nc = Bacc()

    xs = nc.dram_tensor("xs", [CP, B, 2, HW], F16, kind="ExternalInput")
    # stacked conv weights, contiguous per partition:
    # [cp, 4 = (f_lo,f_hi,t_lo,t_hi), par, 128]
    wpack = nc.dram_tensor("wpack", [CP, 4, 2, 128], F16, kind="ExternalInput")
    ident = nc.dram_tensor("ident", [128, 128], F32, kind="ExternalInput")
    # cols 0:V = one-hot(to) on rows 0-63; cols V:2V = promo broadcast
    combo = nc.dram_tensor("combo", [128, 2 * V], F16, kind="ExternalInput")
    out = nc.dram_tensor("out", [B, V], F16, kind="ExternalOutput")

    with tile.TileContext(nc) as tc:
        with (
            tc.tile_pool(name="const", bufs=1) as const,
            tc.tile_pool(name="x2p", bufs=2) as x2p,
            tc.tile_pool(name="ft", bufs=2) as ftp,
            tc.tile_pool(name="psmm", bufs=2, space="PSUM") as psmm,
            tc.tile_pool(name="psgt", bufs=1, space="PSUM") as psgt,
            tc.tile_pool(name="pssc", bufs=2, space="PSUM") as pssc,
        ):
            # ---- persistent tiles ----
            xall = const.tile([CP, B, 2, HW], F16)  # all 8 groups
            wall = const.tile([CP, 4, 2, 128], F16)
            ident_sb = const.tile([128, 128], F32)
            combo_sb = const.tile([128, 2 * V], F16)
            gt_sb = const.tile([D, B, D], F16)  # [j, b, i]
            z = const.tile([128, B], F32)  # [cp, b] partial x^2 sums
            final_sb = const.tile([128, V], F16)
            inv_sb = const.tile([128, 1], F32)

            # ---- head: issue every input DMA up front ----
            # sync HWDGE queue: x groups 0,2,4,6 (SP engine is otherwise idle)
            for g in (0, 2, 4, 6):
                nc.sync.dma_start(
                    out=xall[:, g * GB : (g + 1) * GB, :, :],
                    in_=xs[:, g * GB : (g + 1) * GB, :, :],
                )
            # scalar HWDGE queue: wpack first (it gates the first GEMM),
            # then x groups 1,3 -- ACT is free until the first eviction
            nc.scalar.dma_start(out=wall, in_=wpack[:, :, :, :])
            for g in (1, 3):
                nc.scalar.dma_start(
                    out=xall[:, g * GB : (g + 1) * GB, :, :],
                    in_=xs[:, g * GB : (g + 1) * GB, :, :],
                )
            # gpsimd SWDGE queue: x groups 5,7 + late-needed constants.
            # GPSIMD's first reduction isn't due until ~11us in, so the
            # descriptor-generation time is free here.
            for g in (5, 7):
                nc.gpsimd.dma_start(
                    out=xall[:, g * GB : (g + 1) * GB, :, :],
                    in_=xs[:, g * GB : (g + 1) * GB, :, :],
                )
            nc.gpsimd.dma_start(out=ident_sb, in_=ident[:, :])
            nc.gpsimd.dma_start(out=combo_sb, in_=combo[:, :])

            # score psum chunks (column-partitioned; 2-buf rotation, so
            # chunk q+2 reuses chunk q's bank after its finalize)
            n_chunks = len(CHUNK_BOUNDS) - 1
            zt_ps = pssc.tile([128, 512], F32, tag="sc")  # z transpose target
            sc_ps = []
            for _q in range(n_chunks):
                sc_chunk = pssc.tile([128, 512], F32, tag="sc")
                sc_ps.append(sc_chunk)

            # PE warm-up burst while waiting for group 0's x: the HAM clock
            # gate needs ~3.4us of sustained matmul activity to lift the PE
            # from 1.2 to 2.4 GHz. A memset tile (no DMA dependency) lets
            # the burst start right after the preamble.
            wu_w = const.tile([128, 128], F16)
            nc.vector.memset(wu_w, 0.25)
            wu_ps = psgt.tile([D, 2, PAIRS_PER_GROUP, D], F32, tag="g2")
            for k in range(WARMUP_MM):
                nc.tensor.matmul(
                    out=wu_ps[:, 0, k % PAIRS_PER_GROUP, :],
                    lhsT=wu_w[:, 0:64],
                    rhs=wu_w[:, 64:128],
                    start=True,
                    stop=True,
                    tile_position=(0, 0),
                )

            # ---- main loop over batch groups ----
            for g in range(NGROUPS):
                b0 = g * GB
                xt = xall[:, b0 : b0 + GB, :, :]
                last = g == NGROUPS - 1

                # x^2: one DVE square over the whole group; GPSIMD folds the
                # two halves (its only fast free-axis op is tensor_add);
                # DVE reduces the folded [GB, 64] half into z.
                x2t = x2p.tile([128, GB, 2 * HW], F16, tag="x2")
                xflat = xt.rearrange("p b par hw -> p b (par hw)")
                nc.vector.tensor_mul(out=x2t[:, :, :], in0=xflat, in1=xflat)
                xh = x2p.tile([128, GB, HW], F16, tag="xh")
                kb = GP_F1_B
                if kb > 0:
                    nc.gpsimd.tensor_add(
                        out=xh[:, 0:kb, :],
                        in0=x2t[:, 0:kb, 0:HW],
                        in1=x2t[:, 0:kb, HW : 2 * HW],
                    )
                if kb < GB:
                    nc.vector.tensor_add(
                        out=xh[:, kb:GB, :],
                        in0=x2t[:, kb:GB, 0:HW],
                        in1=x2t[:, kb:GB, HW : 2 * HW],
                    )
                nc.vector.tensor_reduce(
                    out=z[:, b0 : b0 + GB],
                    in_=xh[:, :, :],
                    axis=mybir.AxisListType.X,
                    op=mybir.AluOpType.add,
                )

                # GEMM: psum rows 0-63 = even-batch d, rows 64-127 = odd-batch d
                xv = xt.rearrange("p (pr two) par hw -> p pr two par hw", two=2)
                ps2 = psmm.tile([128, 2, PAIRS_PER_GROUP, HW], F32, tag="ps2")
                for fi, w0 in ((0, 0), (1, 2)):
                    for mi in range(4):
                        half, par0 = mi // 2, mi % 2
                        nc.tensor.matmul(
                            out=ps2[:, fi, :, :],
                            lhsT=wall[:, w0 + half, par0, :],
                            rhs=xv[:, :, half, par0, :],
                            start=(mi == 0),
                            stop=(mi == 3),
                        )
                ft_sb = ftp.tile([128, 2, PAIRS_PER_GROUP, HW], F16, tag="ft")
                if last and SPLIT_LAST:
                    # the end-of-loop chain (GEMM -> ft evict -> Gt ->
                    # gt evict -> score) is latency-critical: split both
                    # evicts across ACT and DVE
                    nc.scalar.copy(out=ft_sb[:, 0, :, :], in_=ps2[:, 0, :, :])
                    nc.vector.tensor_copy(out=ft_sb[:, 1, :, :], in_=ps2[:, 1, :, :])
                else:
                    nc.scalar.copy(out=ft_sb[:, :, :, :], in_=ps2[:, :, :, :])

                # pair-packed Gt matmuls: Gt_b[j, i] = sum_d t[d,j] f[d,i]
                # The two row groups MUST write different psum banks:
                # concurrent row-tiled PE writes to one bank kill the HW run.
                pgt2 = psgt.tile([D, 2, PAIRS_PER_GROUP, D], F32, tag="g2")
                for w in range(PAIRS_PER_GROUP):
                    nc.tensor.matmul(
                        out=pgt2[:, 0, w, :],
                        lhsT=ft_sb[0:64, 1, w, :],
                        rhs=ft_sb[0:64, 0, w, :],
                        start=True,
                        stop=True,
                        tile_position=(0, 0),
                    )
                    nc.tensor.matmul(
                        out=pgt2[:, 1, w, :],
                        lhsT=ft_sb[64:128, 1, w, :],
                        rhs=ft_sb[64:128, 0, w, :],
                        start=True,
                        stop=True,
                        tile_position=(64, 0),
                    )
                # evict [j, (q, pair), i] -> gt[j, b, i], b = 2*(g*8+pr)+q
                # (contiguous 64-elem inner runs)
                if last and SPLIT_LAST:
                    hp = PAIRS_PER_GROUP // 2
                    nc.scalar.copy(
                        out=gt_sb[:, b0 : b0 + GB // 2, :].rearrange(
                            "j (pr q) i -> j q pr i", q=2
                        ),
                        in_=pgt2[:, :, 0:hp, :],
                    )
                    nc.vector.tensor_copy(
                        out=gt_sb[:, b0 + GB // 2 : b0 + GB, :].rearrange(
                            "j (pr q) i -> j q pr i", q=2
                        ),
                        in_=pgt2[:, :, hp:, :],
                    )
                else:
                    nc.scalar.copy(
                        out=gt_sb[:, b0 : b0 + GB, :].rearrange(
                            "j (pr q) i -> j q pr i", q=2
                        ),
                        in_=pgt2[:, :, :, :],
                    )

            # ---- 1/rms2 per batch (overlaps the score matmuls) ----
            nc.tensor.transpose(out=zt_ps[:, 0:128], in_=z[:, :], identity=ident_sb[:, :])
            nc.vector.tensor_reduce(
                out=inv_sb[:, :],
                in_=zt_ps[:, 0:128],
                axis=mybir.AxisListType.X,
                op=mybir.AluOpType.add,
            )
            nc.vector.tensor_scalar(
                out=inv_sb[:, :],
                in0=inv_sb[:, :],
                scalar1=1.0 / (C * HW),
                scalar2=EPS,
                op0=mybir.AluOpType.mult,
                op1=mybir.AluOpType.add,
            )
            nc.vector.reciprocal(out=inv_sb[:, :], in_=inv_sb[:, :])

            # ---- one-hot score matmuls (columns in from_idx-sorted order) ----
            for i, col0, ncols in seg_plan:
                q = next(
                    k for k in range(n_chunks) if CHUNK_BOUNDS[k + 1] > col0
                )
                c0 = col0 - CHUNK_BOUNDS[q]
                nc.tensor.matmul(
                    out=sc_ps[q][:, c0 : c0 + ncols],
                    lhsT=gt_sb[:, :, i],
                    rhs=combo_sb[0:64, col0 : col0 + ncols],
                    start=True,
                    stop=True,
                )

            # ---- fused finalize: out = score * inv[b] + promo_sorted ----
            for q in range(n_chunks):
                q0 = CHUNK_BOUNDS[q]
                n = CHUNK_BOUNDS[q + 1] - q0
                nc.vector.scalar_tensor_tensor(
                    out=final_sb[:, q0 : q0 + n],
                    in0=sc_ps[q][:, 0:n],
                    scalar=inv_sb[:, 0:1],
                    in1=combo_sb[:, V + q0 : V + q0 + n],
                    op0=mybir.AluOpType.mult,
                    op1=mybir.AluOpType.add,
                )
                # per-chunk store so the DMA overlaps later chunks' finalize
                dma_eng = nc.sync if q % 2 == 0 else nc.scalar
                dma_eng.dma_start(
                    out=out[:, q0 : q0 + n],
                    in_=final_sb[:, q0 : q0 + n],
                )

    nc.compile()
    return nc


_NC_CACHE = {}


def _plan_from_indices(from_idx, to_idx):
    from_idx = np.asarray(from_idx, np.int64)
    to_idx = np.asarray(to_idx, np.int64)
    order = np.argsort(from_idx, kind="stable")
    fi_sorted = from_idx[order]
    seg_plan = []
    col = 0
    for i in range(HW):
        n = int(np.count_nonzero(fi_sorted == i))
        while n > 0:
            # segments must not cross a psum chunk boundary
            bound = next(b for b in CHUNK_BOUNDS[1:] if b > col)
            m = min(n, bound - col)
            seg_plan.append((i, col, m))
            col += m
            n -= m
    assert col == V
    onehot = np.zeros((D, V), np.float16)
    onehot[to_idx[order], np.arange(V)] = 1.0
    return tuple(seg_plan), onehot, order


def _host_inputs(from_w, to_w):
    def stack_w(wmat):
        wt = np.ascontiguousarray(wmat.T).reshape(CP, 2, D)  # [cp, par, d]
        lo = np.zeros((2, CP, 128), np.float16)
        hi = np.zeros((2, CP, 128), np.float16)
        lo[:, :, 0:D] = wt.transpose(1, 0, 2)
        hi[:, :, D:128] = wt.transpose(1, 0, 2)
        return lo, hi

    wf_lo, wf_hi = stack_w(np.asarray(from_w, np.float32))
    wt_lo, wt_hi = stack_w(np.asarray(to_w, np.float32))
    return wf_lo, wf_hi, wt_lo, wt_hi


def _device_inputs(x, from_w, to_w, promo_bias, from_idx, to_idx, promo_idx):
    """Build (seg_plan, shared input map, per-core xs list, unsort order)."""
    seg_plan, onehot, order = _plan_from_indices(from_idx, to_idx)
    wf_lo, wf_hi, wt_lo, wt_hi = _host_inputs(from_w, to_w)
    # [4, 2, CP, 128] -> [CP, 4, 2, 128] contiguous so the upload DMA is
    # one 2KB descriptor per partition
    wpack = np.ascontiguousarray(
        np.stack([wf_lo, wf_hi, wt_lo, wt_hi], axis=0).transpose(2, 0, 1, 3)
    )
    promo = np.asarray(promo_bias, np.float32)[np.asarray(promo_idx, np.int64)]
    combo = np.zeros((128, 2 * V), np.float16)
    combo[0:D, 0:V] = onehot
    combo[:, V : 2 * V] = promo[order].astype(np.float16)[None, :]
    shared = {
        "wpack": wpack,
        "ident": np.eye(128, dtype=np.float32),
        "combo": combo,
    }
    # x [B_TOT, C, HW] -> per-core [cp, b, par, hw] fp16 (4KB contiguous
    # per partition per group DMA)
    xr = np.asarray(x, np.float32).reshape(B_TOT, C, HW)
    xs_list = []
    for c in range(N_CORES):
        xc = xr[c * B : (c + 1) * B].reshape(B, CP, 2, HW)
        xs_list.append(np.ascontiguousarray(xc.transpose(1, 0, 2, 3)).astype(np.float16))
    return seg_plan, shared, xs_list, order


def kernel(
    x,
    norm_weight,
    from_w,
    from_b,
    to_w,
    to_b,
    promo_bias,
    from_idx,
    to_idx,
    promo_idx,
):
    x = np.asarray(x, np.float32)
    norm_weight = np.asarray(norm_weight, np.float32)
    from_b = np.asarray(from_b, np.float32)
    to_b = np.asarray(to_b, np.float32)

    if (
        np.any(from_b != 0.0)
        or np.any(to_b != 0.0)
        or not np.allclose(norm_weight, 1.0)
    ):
        # General-correctness fallback; never hit for this problem's input
        # distribution (norm_weight is ones, conv biases are zeros).
        return _host_reference(
            x, norm_weight, from_w, from_b, to_w, to_b, promo_bias,
            from_idx, to_idx, promo_idx,
        )

    seg_plan, shared, xs_list, order = _device_inputs(
        x, from_w, to_w, promo_bias, from_idx, to_idx, promo_idx
    )
    if seg_plan not in _NC_CACHE:
        _NC_CACHE[seg_plan] = build_kernel(seg_plan)
    nc = _NC_CACHE[seg_plan]

    in_maps = [dict(shared, xs=xs_list[c]) for c in range(N_CORES)]
    res = run_bass_kernel_spmd(nc, in_maps, core_ids=list(range(N_CORES)))
    full = np.empty((B_TOT, V), np.float32)
    for c in range(N_CORES):
        dev = np.asarray(res.results[c]["out"], np.float32)  # sorted columns
        full[c * B : (c + 1) * B, order] = dev
    return full


def _host_reference(
    x, norm_weight, from_w, from_b, to_w, to_b, promo_bias, from_idx, to_idx, promo_idx
):
    b, c, w, h = x.shape
    rms = np.sqrt(np.mean(x * x, axis=(1, 2, 3), keepdims=True) + EPS)
    xn = (x / rms) * norm_weight[None]
    f = (
        np.einsum("bchw,dc->bdhw", xn, from_w) + from_b[None, :, None, None]
    ).reshape(b, -1, w * h)
    t = (
        np.einsum("bchw,dc->bdhw", xn, to_w) + to_b[None, :, None, None]
    ).reshape(b, -1, w * h)
    score = np.einsum("bdv,bdv->bv", f[:, :, from_idx], t[:, :, to_idx])
    return (score + promo_bias[promo_idx][None, :]).astype(np.float32)


# revision 9
# speedup vs baseline: 1.0193x; 1.0193x over previous
"""Trainium2 Bass kernel for nn_BilinearHead (RMSNorm -> two 1x1 convs ->
bilinear scores at fixed index pairs + promo bias).

Math (per batch b):
    rms2[b]    = mean(x[b]**2) + eps
    f[b]       = from_w @ (x[b] * norm_weight) ;  t[b] = to_w @ (...)
    score[b,v] = <f[b,:,from_idx[v]], t[b,:,to_idx[v]]> / rms2[b]
                 + promo_bias[promo_idx[v]]
(valid because norm_weight == 1 and the conv biases are 0 for this problem's
input distribution; kernel() verifies and falls back to a host reference
otherwise).

Device algorithm (pure data parallel over batch: 8 cores x 128 batches),
all-fp16 on device:

  1. Host pre-packs x as fp16 [cp=128, b=128, par=2, hw=64] (4KB contiguous
     per partition per group DMA) and wpack as [cp, 4, 2, 128] (contiguous
     2KB/partition DMA -- a scattered layout here cost ~5us and gated the
     first GEMM in earlier versions).
  2. x group loads are spread across the sync/scalar HWDGE queues and the
     gpsimd SWDGE queue, all issued at the head so DMA streams continuously.
  3. Per batch-group of 16: one DVE square over [128, 2048], then the
     per-(cp,b) reduction split between GPSIMD (otherwise idle) and DVE.
  4. PE GEMM (fp16, parity-packed stacked weights): psum rows 0-63 =
     even-batch d, 64-127 = odd-batch d -> f, t in adjacent psum banks;
     single full-lane ACT evict.
  5. PE pair-packed Gt matmuls (quadrants (0,0)/(64,0), separate psum
     banks) -> ACT-evict to gt[64 j, 128 b, 64 i].
  6. After the loop: PE transpose z -> DVE reduce/scale/recip -> inv[b],
     overlapping the score matmuls.
  7. PE one-hot score matmuls per distinct from_idx value (columns sorted
     by from_idx); fused finalize per psum chunk on DVE
     (scalar_tensor_tensor: score * inv[b] + promo) -> fp16 -> DMA out.
  8. Host un-sorts columns and casts fp32.
"""

import sys

sys.path.insert(0, "/opt/trn_rl_repo")

import numpy as np

import concourse.bass as bass
import concourse.tile as tile
from concourse import mybir
from concourse.bacc import Bacc
from concourse.bass_utils import run_bass_kernel_spmd

# Problem shape (hardcoded per contest contract)
B_TOT, C, HW, D, V = 1024, 256, 64, 64, 1968
N_CORES = 8
B = B_TOT // N_CORES  # 128 batches per core
CP = C // 2  # 128 channel pairs (partition dim for GEMM)
NGROUPS = 8
GB = B // NGROUPS  # 16 batches per group
PAIRS_PER_GROUP = GB // 2
EPS = 1e-6
# score psum chunk boundaries (<=512 per bank); the last chunk is tiny so
# the final finalize+store tail after the last score matmul is short
CHUNK_BOUNDS = [0, 512, 1024, 1536, 1904, V]
F32 = mybir.dt.float32
F16 = mybir.dt.float16

# ---- engine-assignment knobs ----
# x^2 path: DVE squares the whole group, GPSIMD (otherwise idle) does the
# first fold-add (GPSIMD has no free-axis reduce, but tensor_add works),
# DVE reduces the folded half. GP_F1_B = batches folded on GPSIMD.
# The whole x^2 pipeline is emitted BEFORE the GEMM loop: it is paced by
# x-group DMA arrivals, not by the GEMM pipeline, and DVE's reduce of
# group g is emitted after group g+1's square so DVE never stalls on
# GPSIMD's fold (in-order queues).
GP_F1_B = 16
# split the last group's evictions across ACT and DVE (DVE is idle by the
# end of the loop) to cut the end-of-loop latency chain
SPLIT_LAST = True
# warmup matmuls (64-col) to lift the HAM clock gate while x0 loads
WARMUP_MM = 44


def build_kernel(seg_plan):
    """seg_plan: list of (i, col0, ncols) score-matmul segments, where i is
    the from_idx value, col0 the starting column in from_idx-sorted order,
    and the segment does not cross a 512 psum-bank boundary."""
    nc = Bacc()

    xs = nc.dram_tensor("xs", [CP, B, 2, HW], F16, kind="ExternalInput")
    # stacked conv weights, contiguous per partition:
    # [cp, 4 = (f_lo,f_hi,t_lo,t_hi), par, 128]
    wpack = nc.dram_tensor("wpack", [CP, 4, 2, 128], F16, kind="ExternalInput")
    ident = nc.dram_tensor("ident", [128, 128], F32, kind="ExternalInput")
    # cols 0:V = one-hot(to) on rows 0-63; cols V:2V = promo broadcast
    combo = nc.dram_tensor("combo", [128, 2 * V], F16, kind="ExternalInput")
    out = nc.dram_tensor("out", [B, V], F16, kind="ExternalOutput")

    with tile.TileContext(nc) as tc:
        with (
            tc.tile_pool(name="const", bufs=1) as const,
            tc.tile_pool(name="x2p", bufs=2) as x2p,
            tc.tile_pool(name="ft", bufs=2) as ftp,
            tc.tile_pool(name="psmm", bufs=2, space="PSUM") as psmm,
            tc.tile_pool(name="psgt", bufs=1, space="PSUM") as psgt,
            tc.tile_pool(name="pssc", bufs=2, space="PSUM") as pssc,
        ):
            # ---- persistent tiles ----
            xall = const.tile([CP, B, 2, HW], F16)  # all 8 groups
            wall = const.tile([CP, 4, 2, 128], F16)
            ident_sb = const.tile([128, 128], F32)
            combo_sb = const.tile([128, 2 * V], F16)
            gt_sb = const.tile([D, B, D], F16)  # [j, b, i]
            z = const.tile([128, B], F32)  # [cp, b] partial x^2 sums
            final_sb = const.tile([128, V], F16)
            inv_sb = const.tile([128, 1], F32)

            # ---- head: issue every input DMA up front ----
            # sync HWDGE queue: x groups 0,2,4,6 (SP engine is otherwise idle)
            for g in (0, 2, 4, 6):
                nc.sync.dma_start(
                    out=xall[:, g * GB : (g + 1) * GB, :, :],
                    in_=xs[:, g * GB : (g + 1) * GB, :, :],
                )
            # scalar HWDGE queue: wpack first (it gates the first GEMM),
            # then x groups 1,3 -- ACT is free until the first eviction
            nc.scalar.dma_start(out=wall, in_=wpack[:, :, :, :])
            for g in (1, 3):
                nc.scalar.dma_start(
                    out=xall[:, g * GB : (g + 1) * GB, :, :],
                    in_=xs[:, g * GB : (g + 1) * GB, :, :],
                )
            # gpsimd SWDGE queue: x groups 5,7 + late-needed constants.
            # GPSIMD's first reduction isn't due until ~11us in, so the
            # descriptor-generation time is free here.
            nc.gpsimd.dma_start(out=ident_sb, in_=ident[:, :])
            for g in (5, 7):
                nc.gpsimd.dma_start(
                    out=xall[:, g * GB : (g + 1) * GB, :, :],
                    in_=xs[:, g * GB : (g + 1) * GB, :, :],
                )
            nc.gpsimd.dma_start(out=combo_sb, in_=combo[:, :])

            # score psum chunks (column-partitioned; 2-buf rotation, so
            # chunk q+2 reuses chunk q's bank after its finalize)
            n_chunks = len(CHUNK_BOUNDS) - 1
            zt_ps = pssc.tile([128, 512], F32, tag="sc")  # z transpose target
            sc_ps = []
            for _q in range(n_chunks):
                sc_chunk = pssc.tile([128, 512], F32, tag="sc")
                sc_ps.append(sc_chunk)

            # PE warm-up burst while waiting for group 0's x: the HAM clock
            # gate needs ~3.4us of sustained matmul activity to lift the PE
            # from 1.2 to 2.4 GHz. A memset tile (no DMA dependency) lets
            # the burst start right after the preamble.
            wu_w = const.tile([128, 128], F16)
            nc.vector.memset(wu_w, 0.25)
            wu_ps = psgt.tile([D, 2, PAIRS_PER_GROUP, D], F32, tag="g2")
            for k in range(WARMUP_MM):
                nc.tensor.matmul(
                    out=wu_ps[:, 0, k % PAIRS_PER_GROUP, :],
                    lhsT=wu_w[:, 0:64],
                    rhs=wu_w[:, 64:128],
                    start=True,
                    stop=True,
                    tile_position=(0, 0),
                )

            # ---- x^2 pipeline (DVE + GPSIMD), decoupled from the GEMM loop.
            # Paced by x-group arrivals; the reduce of group g is emitted
            # after group g+1's square so DVE never stalls on GPSIMD.
            def emit_sq(g):
                b0 = g * GB
                x2t = x2p.tile([128, GB, 2 * HW], F16, tag="x2", bufs=3)
                xflat = xall[:, b0 : b0 + GB, :, :].rearrange(
                    "p b par hw -> p b (par hw)"
                )
                nc.vector.tensor_mul(out=x2t[:, :, :], in0=xflat, in1=xflat)
                xh = x2p.tile([128, GB, HW], F16, tag="xh", bufs=2)
                kb = GP_F1_B
                if kb > 0:
                    nc.gpsimd.tensor_add(
                        out=xh[:, 0:kb, :],
                        in0=x2t[:, 0:kb, 0:HW],
                        in1=x2t[:, 0:kb, HW : 2 * HW],
                    )
                if kb < GB:
                    nc.vector.tensor_add(
                        out=xh[:, kb:GB, :],
                        in0=x2t[:, kb:GB, 0:HW],
                        in1=x2t[:, kb:GB, HW : 2 * HW],
                    )
                return xh

            def emit_red(g, xh):
                nc.vector.tensor_reduce(
                    out=z[:, g * GB : (g + 1) * GB],
                    in_=xh[:, :, :],
                    axis=mybir.AxisListType.X,
                    op=mybir.AluOpType.add,
                )

            xhs = {}
            for g in range(NGROUPS):
                xhs[g] = emit_sq(g)
                if g >= 1:
                    emit_red(g - 1, xhs[g - 1])
            emit_red(NGROUPS - 1, xhs[NGROUPS - 1])

            # ---- main GEMM loop over batch groups (PE + ACT only) ----
            for g in range(NGROUPS):
                b0 = g * GB
                xt = xall[:, b0 : b0 + GB, :, :]
                last = g == NGROUPS - 1

                # GEMM: psum rows 0-63 = even-batch d, rows 64-127 = odd-batch d
                xv = xt.rearrange("p (pr two) par hw -> p pr two par hw", two=2)
                ps2 = psmm.tile([128, 2, PAIRS_PER_GROUP, HW], F32, tag="ps2")
                for fi, w0 in ((0, 0), (1, 2)):
                    for mi in range(4):
                        half, par0 = mi // 2, mi % 2
                        nc.tensor.matmul(
                            out=ps2[:, fi, :, :],
                            lhsT=wall[:, w0 + half, par0, :],
                            rhs=xv[:, :, half, par0, :],
                            start=(mi == 0),
                            stop=(mi == 3),
                        )
                ft_sb = ftp.tile([128, 2, PAIRS_PER_GROUP, HW], F16, tag="ft")
                if last and SPLIT_LAST:
                    # the end-of-loop chain (GEMM -> ft evict -> Gt ->
                    # gt evict -> score) is latency-critical: split both
                    # evicts across ACT and DVE
                    nc.scalar.copy(out=ft_sb[:, 0, :, :], in_=ps2[:, 0, :, :])
                    nc.vector.tensor_copy(out=ft_sb[:, 1, :, :], in_=ps2[:, 1, :, :])
                else:
                    nc.scalar.copy(out=ft_sb[:, :, :, :], in_=ps2[:, :, :, :])

                # pair-packed Gt matmuls: Gt_b[j, i] = sum_d t[d,j] f[d,i]
                # The two row groups MUST write different psum banks:
                # concurrent row-tiled PE writes to one bank kill the HW run.
                pgt2 = psgt.tile([D, 2, PAIRS_PER_GROUP, D], F32, tag="g2")
                for w in range(PAIRS_PER_GROUP):
                    nc.tensor.matmul(
                        out=pgt2[:, 0, w, :],
                        lhsT=ft_sb[0:64, 1, w, :],
                        rhs=ft_sb[0:64, 0, w, :],
                        start=True,
                        stop=True,
                        tile_position=(0, 0),
                    )
                    nc.tensor.matmul(
                        out=pgt2[:, 1, w, :],
                        lhsT=ft_sb[64:128, 1, w, :],
                        rhs=ft_sb[64:128, 0, w, :],
                        start=True,
                        stop=True,
                        tile_position=(64, 0),
                    )
                # evict [j, (q, pair), i] -> gt[j, b, i], b = 2*(g*8+pr)+q
                # (contiguous 64-elem inner runs)
                if last and SPLIT_LAST:
                    hp = PAIRS_PER_GROUP // 2
                    nc.scalar.copy(
                        out=gt_sb[:, b0 : b0 + GB // 2, :].rearrange(
                            "j (pr q) i -> j q pr i", q=2
                        ),
                        in_=pgt2[:, :, 0:hp, :],
                    )
                    nc.vector.tensor_copy(
                        out=gt_sb[:, b0 + GB // 2 : b0 + GB, :].rearrange(
                            "j (pr q) i -> j q pr i", q=2
                        ),
                        in_=pgt2[:, :, hp:, :],
                    )
                else:
                    nc.scalar.copy(
                        out=gt_sb[:, b0 : b0 + GB, :].rearrange(
                            "j (pr q) i -> j q pr i", q=2
                        ),
                        in_=pgt2[:, :, :, :],
                    )

            # ---- 1/rms2 per batch (overlaps the score matmuls) ----
            nc.tensor.transpose(out=zt_ps[:, 0:128], in_=z[:, :], identity=ident_sb[:, :])
            nc.vector.tensor_reduce(
                out=inv_sb[:, :],
                in_=zt_ps[:, 0:128],
                axis=mybir.AxisListType.X,
                op=mybir.AluOpType.add,
            )
            nc.vector.tensor_scalar(
                out=inv_sb[:, :],
                in0=inv_sb[:, :],
                scalar1=1.0 / (C * HW),
                scalar2=EPS,
                op0=mybir.AluOpType.mult,
                op1=mybir.AluOpType.add,
            )
            nc.vector.reciprocal(out=inv_sb[:, :], in_=inv_sb[:, :])

            # ---- one-hot score matmuls (columns in from_idx-sorted order) ----
            for i, col0, ncols in seg_plan:
                q = next(
                    k for k in range(n_chunks) if CHUNK_BOUNDS[k + 1] > col0
                )
                c0 = col0 - CHUNK_BOUNDS[q]
                nc.tensor.matmul(
                    out=sc_ps[q][:, c0 : c0 + ncols],
                    lhsT=gt_sb[:, :, i],
                    rhs=combo_sb[0:64, col0 : col0 + ncols],
                    start=True,
                    stop=True,
                )

            # ---- fused finalize: out = score * inv[b] + promo_sorted ----
            for q in range(n_chunks):
                q0 = CHUNK_BOUNDS[q]
                n = CHUNK_BOUNDS[q + 1] - q0
                nc.vector.scalar_tensor_tensor(
                    out=final_sb[:, q0 : q0 + n],
                    in0=sc_ps[q][:, 0:n],
                    scalar=inv_sb[:, 0:1],
                    in1=combo_sb[:, V + q0 : V + q0 + n],
                    op0=mybir.AluOpType.mult,
                    op1=mybir.AluOpType.add,
                )
                # per-chunk store so the DMA overlaps later chunks' finalize
                dma_eng = nc.sync if q % 2 == 0 else nc.scalar
                dma_eng.dma_start(
                    out=out[:, q0 : q0 + n],
                    in_=final_sb[:, q0 : q0 + n],
                )

    nc.compile()
    return nc


_NC_CACHE = {}


def _plan_from_indices(from_idx, to_idx):
    from_idx = np.asarray(from_idx, np.int64)
    to_idx = np.asarray(to_idx, np.int64)
    order = np.argsort(from_idx, kind="stable")
    fi_sorted = from_idx[order]
    seg_plan = []
    col = 0
    for i in range(HW):
        n = int(np.count_nonzero(fi_sorted == i))
        while n > 0:
            # segments must not cross a psum chunk boundary
            bound = next(b for b in CHUNK_BOUNDS[1:] if b > col)
            m = min(n, bound - col)
            seg_plan.append((i, col, m))
            col += m
            n -= m
    assert col == V
    onehot = np.zeros((D, V), np.float16)
    onehot[to_idx[order], np.arange(V)] = 1.0
    return tuple(seg_plan), onehot, order


def _host_inputs(from_w, to_w):
    def stack_w(wmat):
        wt = np.ascontiguousarray(wmat.T).reshape(CP, 2, D)  # [cp, par, d]
        lo = np.zeros((2, CP, 128), np.float16)
        hi = np.zeros((2, CP, 128), np.float16)
        lo[:, :, 0:D] = wt.transpose(1, 0, 2)
        hi[:, :, D:128] = wt.transpose(1, 0, 2)
        return lo, hi

    wf_lo, wf_hi = stack_w(np.asarray(from_w, np.float32))
    wt_lo, wt_hi = stack_w(np.asarray(to_w, np.float32))
    return wf_lo, wf_hi, wt_lo, wt_hi


def _device_inputs(x, from_w, to_w, promo_bias, from_idx, to_idx, promo_idx):
    """Build (seg_plan, shared input map, per-core xs list, unsort order)."""
    seg_plan, onehot, order = _plan_from_indices(from_idx, to_idx)
    wf_lo, wf_hi, wt_lo, wt_hi = _host_inputs(from_w, to_w)
    # [4, 2, CP, 128] -> [CP, 4, 2, 128] contiguous so the upload DMA is
    # one 2KB descriptor per partition
    wpack = np.ascontiguousarray(
        np.stack([wf_lo, wf_hi, wt_lo, wt_hi], axis=0).transpose(2, 0, 1, 3)
    )
    promo = np.asarray(promo_bias, np.float32)[np.asarray(promo_idx, np.int64)]
    combo = np.zeros((128, 2 * V), np.float16)
    combo[0:D, 0:V] = onehot
    combo[:, V : 2 * V] = promo[order].astype(np.float16)[None, :]
    shared = {
        "wpack": wpack,
        "ident": np.eye(128, dtype=np.float32),
        "combo": combo,
    }
    # x [B_TOT, C, HW] -> per-core [cp, b, par, hw] fp16 (4KB contiguous
    # per partition per group DMA)
    xr = np.asarray(x, np.float32).reshape(B_TOT, C, HW)
    xs_list = []
    for c in range(N_CORES):
        xc = xr[c * B : (c + 1) * B].reshape(B, CP, 2, HW)
        xs_list.append(np.ascontiguousarray(xc.transpose(1, 0, 2, 3)).astype(np.float16))
    return seg_plan, shared, xs_list, order


def kernel(
    x,
    norm_weight,
    from_w,
    from_b,
    to_w,
    to_b,
    promo_bias,
    from_idx,
    to_idx,
    promo_idx,
):
    x = np.asarray(x, np.float32)
    norm_weight = np.asarray(norm_weight, np.float32)
    from_b = np.asarray(from_b, np.float32)
    to_b = np.asarray(to_b, np.float32)

    if (
        np.any(from_b != 0.0)
        or np.any(to_b != 0.0)
        or not np.allclose(norm_weight, 1.0)
    ):
        # General-correctness fallback; never hit for this problem's input
        # distribution (norm_weight is ones, conv biases are zeros).
        return _host_reference(
            x, norm_weight, from_w, from_b, to_w, to_b, promo_bias,
            from_idx, to_idx, promo_idx,
        )

    seg_plan, shared, xs_list, order = _device_inputs(
        x, from_w, to_w, promo_bias, from_idx, to_idx, promo_idx
    )
    if seg_plan not in _NC_CACHE:
        _NC_CACHE[seg_plan] = build_kernel(seg_plan)
    nc = _NC_CACHE[seg_plan]

    in_maps = [dict(shared, xs=xs_list[c]) for c in range(N_CORES)]
    res = run_bass_kernel_spmd(nc, in_maps, core_ids=list(range(N_CORES)))
    full = np.empty((B_TOT, V), np.float32)
    for c in range(N_CORES):
        dev = np.asarray(res.results[c]["out"], np.float32)  # sorted columns
        full[c * B : (c + 1) * B, order] = dev
    return full


def _host_reference(
    x, norm_weight, from_w, from_b, to_w, to_b, promo_bias, from_idx, to_idx, promo_idx
):
    b, c, w, h = x.shape
    rms = np.sqrt(np.mean(x * x, axis=(1, 2, 3), keepdims=True) + EPS)
    xn = (x / rms) * norm_weight[None]
    f = (
        np.einsum("bchw,dc->bdhw", xn, from_w) + from_b[None, :, None, None]
    ).reshape(b, -1, w * h)
    t = (
        np.einsum("bchw,dc->bdhw", xn, to_w) + to_b[None, :, None, None]
    ).reshape(b, -1, w * h)
    score = np.einsum("bdv,bdv->bv", f[:, :, from_idx], t[:, :, to_idx])
    return (score + promo_bias[promo_idx][None, :]).astype(np.float32)
